# revision 25
# baseline (speedup 1.0000x reference)
"""Trainium2 Bass kernel for nn_CausalSelfAttention_5411658793445.

Sharding: queries (token dim) split 8 ways; K/V projection also token-split,
with the current block's roped K / V exchanged via sliced AllGathers so every
core attends over the full kept KV window (prior cache slice + current block).

Per-core device program (identical SPMD program, per-core data):
  1. fp32 projections of the core's 330-token slice (bf16 matmuls):
     kT [d,t] -> rope -> AllGather (2 head-halves); vT direct [t,d] with the
     softmax ones-column baked in -> AllGather (3 head-quarters); qT [d,t].
  2. RMSNorm (partition-dim sum-of-squares via ones-matmul) + RoPE (pair
     components deinterleaved into partition halves by a host-side weight-row
     permutation; the cross-half combine uses a PE half-swap matmul)
  3. Attention, scores-transposed layout: for each head, l-tiles of 128 kept
     positions, processed in PAIRS: PE scores [l,330]x2 into a 2-bank psum ->
     one strided ACT exp (bf16) -> PE (escT as stationary, s-chunks 128/128/74
     for FWL) x [V | ones-column] accumulating numerator AND denominator in a
     single packed psum bank [<=128, 3, 130].  Zero-padded KV rows contribute
     exactly 1.0 to the denominator each and 0 to the numerator -> one
     constant correction.  Pass 1 (prior KV) overlaps the AllGathers; pass 2
     (current KV) follows.
  4. divide, PE-transpose to [d,t], output projection (prefetched weights),
     + bo -> out rows.
"""

import math
from contextlib import ExitStack

import numpy as np
import ml_dtypes

NC = 8
DIM, NH, HD = 1536, 12, 128
HALF = 64
H, W = 22, 40
FRAME = H * W            # 880
S_TOTAL = 2640
SC = S_TOTAL // NC       # 330
SCP = 384                # padded per-core token count (3*128)
EPS = 1e-6
CT = 22
CH = 21
CW = 21
CLT = SCP // 128         # 3 current l-tiles per source core
NK = DIM // 128          # 12 contraction chunks
CHUNKS = [(0, 128), (128, 128), (256, SC - 256)]   # s-chunks 128/128/74

_BF16 = ml_dtypes.bfloat16
_cache: dict = {}


def _build_theta(freqs_angle, cs):
    start_frame = cs // FRAME
    nf = S_TOTAL // FRAME
    t = freqs_angle[start_frame:start_frame + nf, :CT]
    h = freqs_angle[:H, CT:CT + CH]
    w = freqs_angle[:W, CT + CH:CT + CH + CW]
    tf = np.broadcast_to(t[:, None, None, :], (nf, H, W, CT))
    hf = np.broadcast_to(h[None, :, None, :], (nf, H, W, CH))
    wf = np.broadcast_to(w[None, None, :, :], (nf, H, W, CW))
    return np.concatenate([tf, hf, wf], axis=-1).reshape(nf * H * W, HALF)


def _build_program(n_prior, np_pad, n_pads):
    import concourse.bass as bass  # noqa: F401
    import concourse.tile as tile
    from concourse import bacc, mybir
    from concourse.masks import make_identity

    f32 = mybir.dt.float32
    bf16 = mybir.dt.bfloat16
    Act = mybir.ActivationFunctionType
    Alu = mybir.AluOpType

    NPT = np_pad // 128          # prior l-tiles (21)
    sm_scale = 1.0 / math.sqrt(HD)

    nc = bacc.Bacc("TRN2", target_bir_lowering=False, debug=False,
                   num_devices=NC)

    # host-preswizzled inputs
    xsw = nc.dram_tensor("xsw", [128, NK, SC], bf16, kind="ExternalInput").ap()
    thetaT = nc.dram_tensor("thetaT", [HALF, SC], f32, kind="ExternalInput").ap()
    # wq2/wk2: [ki, mh, ko, mi]; wv2/wo2: [ki, ko, n]
    wq = nc.dram_tensor("wq", [128, NH, NK, 128], bf16, kind="ExternalInput").ap()
    wk = nc.dram_tensor("wk", [128, NH, NK, 128], bf16, kind="ExternalInput").ap()
    wv = nc.dram_tensor("wv", [128, NK, DIM], bf16, kind="ExternalInput").ap()
    wo = nc.dram_tensor("wo", [128, NK, DIM], bf16, kind="ExternalInput").ap()
    gq2 = nc.dram_tensor("gq2", [HD, NH], f32, kind="ExternalInput").ap()
    gk2 = nc.dram_tensor("gk2", [HD, NH], f32, kind="ExternalInput").ap()
    bv1 = nc.dram_tensor("bv1", [1, DIM], bf16, kind="ExternalInput").ap()
    bo1 = nc.dram_tensor("bo1", [1, DIM], bf16, kind="ExternalInput").ap()
    bqr = nc.dram_tensor("bqr", [1, DIM], bf16, kind="ExternalInput").ap()
    bkr = nc.dram_tensor("bkr", [1, DIM], bf16, kind="ExternalInput").ap()
    pswT = nc.dram_tensor("pswT", [HD, HD], bf16, kind="ExternalInput").ap()
    priorKT = nc.dram_tensor("priorKT", [NH, HD, np_pad], bf16,
                             kind="ExternalInput").ap()
    # prior V pre-tiled [h, p, lt, 130] with ones column baked at 128
    priorV2 = nc.dram_tensor("priorV2", [NH, 128, np_pad // 128, 130], bf16,
                             kind="ExternalInput").ap()
    out = nc.dram_tensor("out", [SC, DIM], f32, kind="ExternalOutput").ap()

    with tile.TileContext(nc, trace_sim=False) as tc, ExitStack() as ctx:
        consts = ctx.enter_context(tc.tile_pool(name="consts", bufs=1))
        wstr = ctx.enter_context(tc.tile_pool(name="wstr", bufs=3))
        wcp = ctx.enter_context(tc.tile_pool(name="wcp", bufs=14))
        wop = ctx.enter_context(tc.tile_pool(name="wop", bufs=14))
        xpool = ctx.enter_context(tc.tile_pool(name="xpool", bufs=1))
        acts = ctx.enter_context(tc.tile_pool(name="acts", bufs=1))
        sqp = ctx.enter_context(tc.tile_pool(name="sqp", bufs=2))
        csrp = ctx.enter_context(tc.tile_pool(name="csrp", bufs=2))
        kvs = ctx.enter_context(tc.tile_pool(name="kvs", bufs=4))
        escp = ctx.enter_context(tc.tile_pool(name="escp", bufs=3))
        smal = ctx.enter_context(tc.tile_pool(name="smal", bufs=4))
        outp = ctx.enter_context(tc.tile_pool(name="outp", bufs=1))
        dram = ctx.enter_context(tc.tile_pool(name="dram", bufs=1, space="DRAM"))
        pp = ctx.enter_context(tc.tile_pool(name="pp", bufs=2, space="PSUM"))
        psw = ctx.enter_context(tc.tile_pool(name="psw", bufs=2, space="PSUM"))
        pacc = ctx.enter_context(tc.tile_pool(name="pacc", bufs=2, space="PSUM"))

        # ---------- x (hoisted: the very first DMA issued) ----------
        xs = xpool.tile([128, NK, SC], bf16)
        nc.sync.dma_start(xs, xsw)

        # ---------- constants ----------
        _constv_cache = {}

        def constv(val):
            if val not in _constv_cache:
                t = consts.tile([128, 1], f32, name=f"cv_{len(_constv_cache)}")
                nc.vector.memset(t, val)
                _constv_cache[val] = t
            return _constv_cache[val]

        ident = consts.tile([128, 128], f32)
        make_identity(nc, ident)
        ones_col = consts.tile([128, 1], f32)
        nc.vector.memset(ones_col, 1.0)
        ones_row = consts.tile([1, 128], bf16)
        nc.vector.memset(ones_row, 1.0)
        ones_row_f = consts.tile([1, 128], f32)
        nc.vector.memset(ones_row_f, 1.0)
        psw_sb = consts.tile([HD, HD], bf16)
        nc.sync.dma_start(psw_sb, pswT)
        th2 = consts.tile([128, SC], f32)
        nc.sync.dma_start(th2[0:HALF, :], thetaT)
        nc.sync.dma_start(th2[HALF:128, :], thetaT)
        # CC = [cos; cos], SS = [-sin; sin]
        cc = consts.tile([128, SC], f32)
        ss = consts.tile([128, SC], f32)
        nc.scalar.activation(cc, th2, Act.Sin, bias=constv(math.pi / 2.0))
        nc.scalar.activation(ss[0:HALF, :], th2[0:HALF, :], Act.Sin, scale=constv(-1.0)[0:HALF])
        nc.scalar.activation(ss[HALF:128, :], th2[HALF:128, :], Act.Sin)
        gq_sb = consts.tile([HD, NH], f32)
        gk_sb = consts.tile([HD, NH], f32)
        nc.sync.dma_start(gq_sb, gq2)
        nc.sync.dma_start(gk_sb, gk2)
        bv_sb = consts.tile([1, DIM], bf16)
        bo_sb = consts.tile([1, DIM], bf16)
        nc.sync.dma_start(bv_sb, bv1)
        nc.sync.dma_start(bo_sb, bo1)
        bqr_sb = consts.tile([1, DIM], bf16)
        bkr_sb = consts.tile([1, DIM], bf16)
        nc.sync.dma_start(bqr_sb, bqr)
        nc.sync.dma_start(bkr_sb, bkr)
        ones_sc = consts.tile([1, SC], bf16)
        nc.vector.memset(ones_sc, 1.0)

        # ---------- internal DRAM for collectives ----------
        k_cc = [dram.tile([6, HD, SCP], bf16, name=f"kcc{i}") for i in range(2)]
        kg = [dram.tile([NC, 6, HD, SCP], bf16, addr_space="Shared",
                        name=f"kg{i}") for i in range(2)]
        v_cc = [dram.tile([4, 128, CLT, 130], bf16, name=f"vcc{i}")
                for i in range(3)]
        vg = [dram.tile([NC, 4, 128, CLT, 130], bf16, addr_space="Shared",
                        name=f"vg{i}") for i in range(3)]
        rgroups = [list(range(NC))]

        # ---------- projection helper (q / k): [d, t] + norm factors ------
        def qk_projection(w_dram, b_row, g_sb, name):
            raw = acts.tile([128, NH, SC], bf16, tag="raw", name=f"raw_{name}")
            pss = pacc.tile([128, 512], f32, tag="pacc", name=f"pss_{name}")
            for m in range(NH):
                wm = wstr.tile([128, NK, 128], bf16, tag="wm",
                               name=f"wm_{name}_{m}")
                nc.sync.dma_start(wm, w_dram[:, m])
                ps = pp.tile([128, 1024], f32, tag="pp", name=f"pj_{name}_{m}")
                for kk in range(NK):
                    nc.tensor.matmul(
                        ps[:, :SC], wm[:, kk, :], xs[:, kk, :],
                        start=(kk == 0), stop=False)
                # bias via rank-1 update: ps += b_head (x) ones
                nc.tensor.matmul(
                    ps[:, :SC], b_row[:, m * 128:(m + 1) * 128], ones_sc,
                    start=False, stop=True)
                nc.vector.tensor_scalar_mul(raw[:, m, :], ps[:, :SC],
                                            g_sb[:, m:m + 1])
                sq = sqp.tile([128, SC], f32, tag="sq")
                nc.scalar.activation(sq, ps[:, :SC], Act.Square)
                nc.tensor.matmul(pss[0:1, :SC], ones_col, sq,
                                 start=(m == 0), stop=(m == NH - 1))
            r1 = smal.tile([1, SC], f32, tag="r1")
            nc.scalar.activation(r1, pss[0:1, :SC], Act.Sqrt,
                                 scale=constv(1.0 / DIM)[0:1],
                                 bias=constv(EPS)[0:1])
            rr = smal.tile([1, SC], f32, tag="rr")
            nc.vector.reciprocal(rr, r1)
            rrb = psw.tile([128, 512], f32, tag="psw", name=f"rrb_{name}")
            nc.tensor.matmul(rrb[:, :SC], ones_row_f, rr,
                             start=True, stop=True)
            ccr = csrp.tile([128, SC], bf16, tag="ccr")
            ssr = csrp.tile([128, SC], bf16, tag="ssr")
            nc.vector.tensor_mul(ccr, cc, rrb[:, :SC])
            nc.vector.tensor_mul(ssr, ss, rrb[:, :SC])
            return raw, ccr, ssr

        def rope_chunk(raw, ccr, ssr, m, dst_ap, name):
            # dst = raw*ccr + swap_halves(raw)*ssr   (swap via PE matmul).
            # The psum swap result is evacuated to bf16 on ACT (idle in this
            # phase) so all three DVE ops run in the 2x bf16 mode.
            pw = psw.tile([128, 512], f32, tag="psw", name=f"sw_{name}_{m}")
            nc.tensor.matmul(pw[:, :SC], psw_sb, raw[:, m, :],
                             start=True, stop=True)
            pwb = sqp.tile([128, SC], bf16, tag="pwb")
            nc.scalar.copy(pwb, pw[:, :SC])
            m1 = sqp.tile([128, SC], bf16, tag="m1")
            nc.vector.tensor_mul(m1, raw[:, m, :], ccr)
            m2 = sqp.tile([128, SC], bf16, tag="m2")
            nc.vector.tensor_mul(m2, pwb, ssr)
            nc.vector.tensor_add(dst_ap, m1, m2)

        # ---------- K (+ sliced AllGather, 2 head-halves) ----------
        raw_k, ccr_k, ssr_k = qk_projection(wk, bkr_sb, gk_sb, "k")
        kn = acts.tile([128, NH, SCP], bf16)
        nc.vector.memset(kn, 0.0)
        for m in range(NH):
            rope_chunk(raw_k, ccr_k, ssr_k, m, kn[:, m, :SC], "k")
            if m in (5, 11):
                half = 0 if m == 5 else 1
                for m2 in range(6):
                    nc.sync.dma_start(k_cc[half][m2], kn[:, 6 * half + m2, :])
                nc.gpsimd.collective_compute(
                    "AllGather", Alu.bypass, replica_groups=rgroups,
                    ins=[k_cc[half].opt()], outs=[kg[half].opt()])
        # prefetch prior KV for the first heads of pass 1 (DMA is idle-ish
        # during the projection phase; pass-1 then starts compute-bound)
        NPT = np_pad // 128
        pre_kv = []
        for h in range(4):
            pkh = kvs.tile([128, np_pad], bf16, tag="kload", name=f"prek{h}")
            nc.sync.dma_start(pkh, priorKT[h])
            pvh = kvs.tile([128, NPT, 130], bf16, tag="vload", name=f"prev{h}")
            nc.sync.dma_start(pvh, priorV2[h])
            pre_kv.append((pkh, pvh))

        # ---------- V production helper (emitted interleaved with pass 1
        # so its PE work hides under pass-1's ACT-bound exp) -------------
        # vt2[t_part, chunk, head, 130]; col 128 = 1.0 (softmax denominator),
        # pad token rows (beyond chunk width) stay 0 except the ones column.
        vt2 = acts.tile([128, 3, NH, 130], bf16)
        nc.vector.memset(vt2, 0.0)
        nc.vector.memset(vt2[:, :, :, 128:129], 1.0)

        def emit_v_oc(oc):
            wcs = []
            for kk in range(NK):
                wc = wcp.tile([128, 512], bf16, tag="wc", name=f"wv_{oc}_{kk}")
                nc.sync.dma_start(wc, wv[:, kk, oc * 512:(oc + 1) * 512])
                wcs.append(wc)
            for ci, (off, w) in enumerate(CHUNKS):
                pv = pacc.tile([128, 512], f32, tag="pacc",
                               name=f"pv_{oc}_{ci}")
                for kk in range(NK):
                    nc.tensor.matmul(
                        pv[0:w, :], xs[:, kk, off:off + w], wcs[kk],
                        start=(kk == 0), stop=False)
                nc.tensor.matmul(
                    pv[0:w, :], ones_row[:, 0:w],
                    bv_sb[:, oc * 512:(oc + 1) * 512],
                    start=False, stop=True)
                nc.vector.tensor_copy(
                    vt2[0:w, ci, 4 * oc:4 * (oc + 1), 0:128],
                    pv[0:w, :].rearrange("p (h d) -> p h d", h=4))
            # ship this oc's 4 heads (full 128 rows incl. zero padding)
            for hh in range(4):
                h = 4 * oc + hh
                for ci in range(CLT):
                    nc.sync.dma_start(v_cc[oc][hh, :, ci, :], vt2[:, ci, h, :])
            nc.gpsimd.collective_compute(
                "AllGather", Alu.bypass, replica_groups=rgroups,
                ins=[v_cc[oc].opt()], outs=[vg[oc].opt()])

        # ---------- Q ----------
        raw_q, ccr_q, ssr_q = qk_projection(wq, bqr_sb, gq_sb, "q")
        qn = acts.tile([128, NH, SC], bf16)
        for m in range(NH):
            rope_chunk(raw_q, ccr_q, ssr_q, m, qn[:, m, :], "q")

        # ---------- attention ----------
        part1 = outp.tile([128, NH, 3, 130], f32)
        oT = outp.tile([128, NH, SC], bf16)

        def attn_accum(h, lhsT_tiles, v_tiles, n_tiles, phase):
            # single packed psum bank: [s-chunk rows, chunk idx, 129+pad]
            pos = pacc.tile([128, 3, 130], f32, tag="pacc",
                            name=f"po_{phase}_{h}")
            n_mm = 0
            total_mm = n_tiles * 3
            for g0 in range(0, n_tiles, 3):
                gn = min(3, n_tiles - g0)
                # scores for 3 l-tiles packed CONTIGUOUSLY across a 2-bank
                # strip (middle tile's matmul split at the bank boundary) so
                # one ACT exp reads [128, 990] with no per-tile overhead.
                ps = pp.tile([128, 1024], f32, tag="pp",
                             name=f"sc_{phase}_{h}_{g0}")
                esc = escp.tile([128, 990], bf16, tag="esc")
                if gn == 3:
                    nc.tensor.matmul(ps[:, 0:330], lhsT_tiles(g0),
                                     qn[:, h, :], start=True, stop=True)
                    nc.tensor.matmul(ps[:, 330:512], lhsT_tiles(g0 + 1),
                                     qn[:, h, 0:182], start=True, stop=True)
                    nc.tensor.matmul(ps[:, 512:660], lhsT_tiles(g0 + 1),
                                     qn[:, h, 182:330], start=True, stop=True)
                    nc.tensor.matmul(ps[:, 660:990], lhsT_tiles(g0 + 2),
                                     qn[:, h, :], start=True, stop=True)
                    nc.scalar.activation(esc, ps[:, 0:990], Act.Exp,
                                         scale=constv(sm_scale))
                else:
                    for j in range(gn):
                        nc.tensor.matmul(ps[:, 512 * j:512 * j + SC],
                                         lhsT_tiles(g0 + j), qn[:, h, :],
                                         start=True, stop=True)
                        nc.scalar.activation(esc[:, SC * j:SC * (j + 1)],
                                             ps[:, 512 * j:512 * j + SC],
                                             Act.Exp, scale=constv(sm_scale))
                for j in range(gn):
                    lt = g0 + j
                    for ci, (off, w) in enumerate(CHUNKS):
                        # one has_written chain for the whole packed bank
                        nc.tensor.matmul(
                            pos[0:w, ci, 0:129],
                            esc[:, SC * j + off:SC * j + off + w], v_tiles(lt),
                            start=(n_mm == 0), stop=(n_mm == total_mm - 1))
                        n_mm += 1
            if phase == "p":
                nc.vector.tensor_copy(part1[:, h, :, :], pos)
            else:
                nc.vector.tensor_add(part1[:, h, :, :], pos,
                                     part1[:, h, :, :])

        # pass 1: prior KV (overlaps the AllGathers); V-projection chunks are
        # interleaved after the first pass-1 heads so their PE work runs
        # while ACT is busy with pass-1 exps.
        for h in range(NH):
            if h < len(pre_kv):
                pkh, pvh = pre_kv[h]
            else:
                pkh = kvs.tile([128, np_pad], bf16, tag="kload")
                nc.sync.dma_start(pkh, priorKT[h])
                pvh = kvs.tile([128, NPT, 130], bf16, tag="vload")
                nc.sync.dma_start(pvh, priorV2[h])
            attn_accum(
                h,
                lambda lt, pkh=pkh: pkh[:, lt * 128:(lt + 1) * 128],
                lambda lt, pvh=pvh: pvh[:, lt, 0:129],
                NPT, "p")
            if h < 3:
                emit_v_oc(h)

        # prefetch first oc of Wo chunks (used after pass 2)
        wo_t = {}
        for hh in range(NH):
            t = wop.tile([128, 512], bf16, tag="wot", name=f"wo_0_{hh}")
            nc.sync.dma_start(t, wo[:, hh, 0:512])
            wo_t[(0, hh)] = t

        # pass 2: current KV (needs AllGather results)
        for h in range(NH):
            kgh = kvs.tile([128, NC, SCP], bf16, tag="kload")
            nc.sync.dma_start(
                kgh, kg[h // 6][:, h % 6].rearrange("c p t -> p c t"))
            vgh = kvs.tile([128, NC, CLT, 130], bf16, tag="vload")
            nc.sync.dma_start(
                vgh, vg[h // 4][:, h % 4].rearrange("c p lt d -> p c lt d"))
            attn_accum(
                h,
                lambda lt, kgh=kgh: kgh[:, lt // CLT,
                                        (lt % CLT) * 128:(lt % CLT + 1) * 128],
                lambda lt, vgh=vgh: vgh[:, lt // CLT, lt % CLT, 0:129],
                NC * CLT, "c")
            # divide by corrected denominator; transpose to [d, t]
            for ci, (off, w) in enumerate(CHUNKS):
                den = smal.tile([128, 1], f32, tag="den")
                nc.vector.tensor_scalar_add(den[0:w, :],
                                            part1[0:w, h, ci, 128:129],
                                            -float(n_pads))
                rcp = smal.tile([128, 1], f32, tag="rcp")
                nc.vector.reciprocal(rcp[0:w, :], den[0:w, :])
                odiv = sqp.tile([128, 128], f32, tag="odiv")
                nc.vector.tensor_scalar_mul(odiv[0:w, :],
                                            part1[0:w, h, ci, 0:128],
                                            rcp[0:w, 0:1])
                ptr = psw.tile([128, 512], f32, tag="psw",
                               name=f"ptr_{h}_{ci}")
                nc.tensor.transpose(ptr[:, :w], odiv[0:w, :],
                                    ident[0:w, 0:w])
                nc.vector.tensor_copy(oT[:, h, off:off + w], ptr[:, :w])

        # ---------- output projection (oc outer; weights prefetched) ------
        for oc in range(3):
            if oc + 1 < 3:
                for hh in range(NH):
                    t = wop.tile([128, 512], bf16, tag="wot",
                                 name=f"wo_{oc + 1}_{hh}")
                    nc.sync.dma_start(
                        t, wo[:, hh, (oc + 1) * 512:(oc + 2) * 512])
                    wo_t[(oc + 1, hh)] = t
            for ci, (off, w) in enumerate(CHUNKS):
                po = pacc.tile([128, 512], f32, tag="pacc",
                               name=f"pout_{oc}_{ci}")
                for hh in range(NH):
                    nc.tensor.matmul(
                        po[0:w, :], oT[:, hh, off:off + w], wo_t[(oc, hh)],
                        start=(hh == 0), stop=False)
                nc.tensor.matmul(
                    po[0:w, :], ones_row[:, 0:w],
                    bo_sb[:, oc * 512:(oc + 1) * 512],
                    start=False, stop=True)
                ob = sqp.tile([128, 512], f32, tag="ob")
                nc.vector.tensor_copy(ob[0:w, :], po[0:w, :])
                nc.sync.dma_start(
                    out[off:off + w, oc * 512:(oc + 1) * 512], ob[0:w, :])

    nc.compile()
    return nc


def _prep(inputs):
    x = np.asarray(inputs["x"], np.float32)
    freqs_angle = np.asarray(inputs["freqs_angle"], np.float32)
    prior_k = np.asarray(inputs["prior_k"], np.float32)
    prior_v = np.asarray(inputs["prior_v"], np.float32)
    cs = int(np.asarray(inputs["current_start"]))

    block = 3 * FRAME
    block_end = (cs // block + 1) * block
    keep_from = max(0, block_end - 6 * FRAME)
    keep_size = min(cs + S_TOTAL - keep_from, prior_k.shape[0] + S_TOTAL)
    n_prior = keep_size - S_TOTAL
    p0 = prior_k.shape[0] - n_prior
    np_pad = -(-n_prior // 128) * 128
    n_pads = (np_pad - n_prior) + NC * (SCP - SC)

    perm = np.concatenate(
        [h * HD + np.concatenate([np.arange(0, HD, 2), np.arange(1, HD, 2)])
         for h in range(NH)])

    WqT = np.ascontiguousarray(np.asarray(inputs["Wq"], np.float32)[perm].T)
    WkT = np.ascontiguousarray(np.asarray(inputs["Wk"], np.float32)[perm].T)
    WvT = np.ascontiguousarray(np.asarray(inputs["Wv"], np.float32).T)
    WoT = np.ascontiguousarray(np.asarray(inputs["Wo"], np.float32).T)

    # [ki, mh, ko, mi] for q/k; [ki, ko, n] for v/o
    wq2 = np.ascontiguousarray(
        WqT.reshape(NK, 128, NH, 128).transpose(1, 2, 0, 3)).astype(_BF16)
    wk2 = np.ascontiguousarray(
        WkT.reshape(NK, 128, NH, 128).transpose(1, 2, 0, 3)).astype(_BF16)
    wv2 = np.ascontiguousarray(
        WvT.reshape(NK, 128, DIM).transpose(1, 0, 2)).astype(_BF16)
    wo2 = np.ascontiguousarray(
        WoT.reshape(NK, 128, DIM).transpose(1, 0, 2)).astype(_BF16)

    def two(vec, p=None):
        v = np.asarray(vec, np.float32)
        if p is not None:
            v = v[p]
        return np.ascontiguousarray(v.reshape(NH, HD).T)

    gq2 = two(inputs["gq"], perm)
    gk2 = two(inputs["gk"], perm)
    bv1 = np.asarray(inputs["bv"], np.float32).reshape(1, DIM).astype(_BF16)
    bo1 = np.asarray(inputs["bo"], np.float32).reshape(1, DIM).astype(_BF16)
    bqr = np.asarray(inputs["bq"], np.float32)[perm].reshape(1, DIM).astype(_BF16)
    bkr = np.asarray(inputs["bk"], np.float32)[perm].reshape(1, DIM).astype(_BF16)

    pswT = np.zeros((HD, HD), _BF16)
    for r in range(HD):
        pswT[(r + HALF) % HD, r] = 1.0   # lhsT of the half-swap permutation

    theta = _build_theta(freqs_angle, cs)          # [S, 64]
    thetaT = np.ascontiguousarray(theta.T)

    pk = prior_k[p0:].reshape(n_prior, DIM)[:, perm]
    priorKT = np.zeros((DIM, np_pad), np.float32)
    priorKT[:, :n_prior] = pk.T
    priorKT = np.ascontiguousarray(priorKT.reshape(NH, HD, np_pad)).astype(_BF16)
    # prior V pre-tiled [h, p, lt, 130]; col 128 = ones (denominator column)
    NPT = np_pad // 128
    priorV2 = np.zeros((NH, np_pad, 130), np.float32)
    priorV2[:, :n_prior, :HD] = prior_v[p0:].transpose(1, 0, 2)
    priorV2[:, :, 128] = 1.0
    priorV2 = np.ascontiguousarray(
        priorV2.reshape(NH, NPT, 128, 130).transpose(0, 2, 1, 3)).astype(_BF16)

    xT = np.ascontiguousarray(x[0].T).astype(_BF16)              # [DIM, S]

    shared = dict(wq=wq2, wk=wk2, wv=wv2, wo=wo2,
                  gq2=gq2, gk2=gk2, bv1=bv1, bo1=bo1, bqr=bqr, bkr=bkr,
                  pswT=pswT, priorKT=priorKT, priorV2=priorV2)
    in_maps = []
    for c in range(NC):
        m = dict(shared)
        xc = xT[:, c * SC:(c + 1) * SC]                          # [DIM, SC]
        m["xsw"] = np.ascontiguousarray(
            xc.reshape(NK, 128, SC).transpose(1, 0, 2))
        m["thetaT"] = np.ascontiguousarray(thetaT[:, c * SC:(c + 1) * SC])
        in_maps.append(m)
    return in_maps, (n_prior, np_pad, n_pads)


def kernel(**inputs) -> np.ndarray:
    import os
    from concourse.bass_utils import run_bass_kernel_spmd

    in_maps, key = _prep(inputs)
    if key not in _cache:
        _cache[key] = _build_program(*key)
    nc = _cache[key]

    trace = bool(int(os.environ.get("KERNEL_TRACE", "0")))
    try:
        res = run_bass_kernel_spmd(
            nc, in_maps, core_ids=list(range(NC)), trace=trace,
            trace_cores=list(range(NC)) if trace else None)
    except ModuleNotFoundError:
        res = run_bass_kernel_spmd(nc, in_maps, core_ids=list(range(NC)),
                                   trace=False)
    kernel.last_results = res
    outp = np.concatenate([res.results[c]["out"] for c in range(NC)], axis=0)
    return outp[None].astype(np.float32)


# revision 29
# speedup vs baseline: 1.0342x; 1.0342x over previous
"""Trainium2 Bass kernel for nn_CausalSelfAttention_5411658793445.

Sharding: queries (token dim) split 8 ways; K/V projection also token-split,
with the current block's roped K / V exchanged via sliced AllGathers so every
core attends over the full kept KV window (prior cache slice + current block).

Per-core device program (identical SPMD program, per-core data):
  1. fp32 projections of the core's 330-token slice (bf16 matmuls):
     kT [d,t] -> rope -> AllGather (2 head-halves); vT direct [t,d] with the
     softmax ones-column baked in -> AllGather (3 head-quarters); qT [d,t].
  2. RMSNorm (partition-dim sum-of-squares via ones-matmul) + RoPE (pair
     components deinterleaved into partition halves by a host-side weight-row
     permutation; the cross-half combine uses a PE half-swap matmul)
  3. Attention, scores-transposed layout: for each head, l-tiles of 128 kept
     positions, processed in PAIRS: PE scores [l,330]x2 into a 2-bank psum ->
     one strided ACT exp (bf16) -> PE (escT as stationary, s-chunks 128/128/74
     for FWL) x [V | ones-column] accumulating numerator AND denominator in a
     single packed psum bank [<=128, 3, 130].  Zero-padded KV rows contribute
     exactly 1.0 to the denominator each and 0 to the numerator -> one
     constant correction.  Pass 1 (prior KV) overlaps the AllGathers; pass 2
     (current KV) follows.
  4. divide, PE-transpose to [d,t], output projection (prefetched weights),
     + bo -> out rows.
"""

import math
from contextlib import ExitStack

import numpy as np
import ml_dtypes

NC = 8
DIM, NH, HD = 1536, 12, 128
HALF = 64
H, W = 22, 40
FRAME = H * W            # 880
S_TOTAL = 2640
SC = S_TOTAL // NC       # 330
SCP = 384                # padded per-core token count (3*128)
EPS = 1e-6
CT = 22
CH = 21
CW = 21
CLT = SCP // 128         # 3 current l-tiles per source core
NK = DIM // 128          # 12 contraction chunks
CHUNKS = [(0, 128), (128, 128), (256, SC - 256)]   # s-chunks 128/128/74

_BF16 = ml_dtypes.bfloat16
_cache: dict = {}


def _build_theta(freqs_angle, cs):
    start_frame = cs // FRAME
    nf = S_TOTAL // FRAME
    t = freqs_angle[start_frame:start_frame + nf, :CT]
    h = freqs_angle[:H, CT:CT + CH]
    w = freqs_angle[:W, CT + CH:CT + CH + CW]
    tf = np.broadcast_to(t[:, None, None, :], (nf, H, W, CT))
    hf = np.broadcast_to(h[None, :, None, :], (nf, H, W, CH))
    wf = np.broadcast_to(w[None, None, :, :], (nf, H, W, CW))
    return np.concatenate([tf, hf, wf], axis=-1).reshape(nf * H * W, HALF)


def _build_program(n_prior, np_pad, n_pads):
    import concourse.bass as bass  # noqa: F401
    import concourse.tile as tile
    from concourse import bacc, mybir
    from concourse.masks import make_identity

    f32 = mybir.dt.float32
    bf16 = mybir.dt.bfloat16
    Act = mybir.ActivationFunctionType
    Alu = mybir.AluOpType

    NPT = np_pad // 128          # prior l-tiles (21)
    sm_scale = 1.0 / math.sqrt(HD)

    nc = bacc.Bacc("TRN2", target_bir_lowering=False, debug=False,
                   num_devices=NC)

    # host-preswizzled inputs
    xsw = nc.dram_tensor("xsw", [128, NK, SC], bf16, kind="ExternalInput").ap()
    thetaT = nc.dram_tensor("thetaT", [HALF, SC], f32, kind="ExternalInput").ap()
    # wq2/wk2: [ki, mh, ko, mi]; wv2/wo2: [ki, ko, n]
    wq = nc.dram_tensor("wq", [128, NH, NK, 128], bf16, kind="ExternalInput").ap()
    wk = nc.dram_tensor("wk", [128, NH, NK, 128], bf16, kind="ExternalInput").ap()
    wv = nc.dram_tensor("wv", [128, NK, DIM], bf16, kind="ExternalInput").ap()
    wo = nc.dram_tensor("wo", [128, NK, DIM], bf16, kind="ExternalInput").ap()
    gq2 = nc.dram_tensor("gq2", [HD, NH], f32, kind="ExternalInput").ap()
    gk2 = nc.dram_tensor("gk2", [HD, NH], f32, kind="ExternalInput").ap()
    bv1 = nc.dram_tensor("bv1", [1, DIM], bf16, kind="ExternalInput").ap()
    bo1 = nc.dram_tensor("bo1", [1, DIM], bf16, kind="ExternalInput").ap()
    bqr = nc.dram_tensor("bqr", [1, DIM], bf16, kind="ExternalInput").ap()
    bkr = nc.dram_tensor("bkr", [1, DIM], bf16, kind="ExternalInput").ap()
    pswT = nc.dram_tensor("pswT", [HD, HD], bf16, kind="ExternalInput").ap()
    priorKT = nc.dram_tensor("priorKT", [NH, HD, np_pad], bf16,
                             kind="ExternalInput").ap()
    # prior V pre-tiled [h, p, lt, 130] with ones column baked at 128
    priorV2 = nc.dram_tensor("priorV2", [NH, 128, np_pad // 128, 130], bf16,
                             kind="ExternalInput").ap()
    out = nc.dram_tensor("out", [SC, DIM], f32, kind="ExternalOutput").ap()

    with tile.TileContext(nc, trace_sim=False) as tc, ExitStack() as ctx:
        consts = ctx.enter_context(tc.tile_pool(name="consts", bufs=1))
        wstr = ctx.enter_context(tc.tile_pool(name="wstr", bufs=3))
        wcp = ctx.enter_context(tc.tile_pool(name="wcp", bufs=14))
        wop = ctx.enter_context(tc.tile_pool(name="wop", bufs=14))
        xpool = ctx.enter_context(tc.tile_pool(name="xpool", bufs=1))
        acts = ctx.enter_context(tc.tile_pool(name="acts", bufs=1))
        sqp = ctx.enter_context(tc.tile_pool(name="sqp", bufs=2))
        csrp = ctx.enter_context(tc.tile_pool(name="csrp", bufs=2))
        kvs = ctx.enter_context(tc.tile_pool(name="kvs", bufs=3))
        escp = ctx.enter_context(tc.tile_pool(name="escp", bufs=3))
        smal = ctx.enter_context(tc.tile_pool(name="smal", bufs=4))
        outp = ctx.enter_context(tc.tile_pool(name="outp", bufs=1))
        dram = ctx.enter_context(tc.tile_pool(name="dram", bufs=1, space="DRAM"))
        pp = ctx.enter_context(tc.tile_pool(name="pp", bufs=2, space="PSUM"))
        psw = ctx.enter_context(tc.tile_pool(name="psw", bufs=2, space="PSUM"))
        pacc = ctx.enter_context(tc.tile_pool(name="pacc", bufs=2, space="PSUM"))

        # ---------- x (hoisted: the very first DMA issued) ----------
        xs = xpool.tile([128, NK, SC], bf16)
        nc.sync.dma_start(xs, xsw)

        # ---------- constants ----------
        _constv_cache = {}

        def constv(val):
            if val not in _constv_cache:
                t = consts.tile([128, 1], f32, name=f"cv_{len(_constv_cache)}")
                nc.vector.memset(t, val)
                _constv_cache[val] = t
            return _constv_cache[val]

        ident = consts.tile([128, 128], f32)
        make_identity(nc, ident)
        ones_col = consts.tile([128, 1], f32)
        nc.vector.memset(ones_col, 1.0)
        ones_row = consts.tile([1, 128], bf16)
        nc.vector.memset(ones_row, 1.0)
        ones_row_f = consts.tile([1, 128], f32)
        nc.vector.memset(ones_row_f, 1.0)
        psw_sb = consts.tile([HD, HD], bf16)
        nc.sync.dma_start(psw_sb, pswT)
        th2 = consts.tile([128, SC], f32)
        nc.sync.dma_start(th2[0:HALF, :], thetaT)
        nc.sync.dma_start(th2[HALF:128, :], thetaT)
        # CC = [cos; cos], SS = [-sin; sin]
        cc = consts.tile([128, SC], f32)
        ss = consts.tile([128, SC], f32)
        nc.scalar.activation(cc, th2, Act.Sin, bias=constv(math.pi / 2.0))
        nc.scalar.activation(ss[0:HALF, :], th2[0:HALF, :], Act.Sin, scale=constv(-1.0)[0:HALF])
        nc.scalar.activation(ss[HALF:128, :], th2[HALF:128, :], Act.Sin)
        gq_sb = consts.tile([HD, NH], f32)
        gk_sb = consts.tile([HD, NH], f32)
        nc.sync.dma_start(gq_sb, gq2)
        nc.sync.dma_start(gk_sb, gk2)
        bv_sb = consts.tile([1, DIM], bf16)
        bo_sb = consts.tile([1, DIM], bf16)
        nc.sync.dma_start(bv_sb, bv1)
        nc.sync.dma_start(bo_sb, bo1)
        bqr_sb = consts.tile([1, DIM], bf16)
        bkr_sb = consts.tile([1, DIM], bf16)
        nc.sync.dma_start(bqr_sb, bqr)
        nc.sync.dma_start(bkr_sb, bkr)
        ones_sc = consts.tile([1, SC], bf16)
        nc.vector.memset(ones_sc, 1.0)

        # ---------- internal DRAM for collectives ----------
        k_cc = [dram.tile([6, HD, SCP], bf16, name=f"kcc{i}") for i in range(2)]
        kg = [dram.tile([NC, 6, HD, SCP], bf16, addr_space="Shared",
                        name=f"kg{i}") for i in range(2)]
        v_cc = [dram.tile([4, 128, CLT, 130], bf16, name=f"vcc{i}")
                for i in range(3)]
        vg = [dram.tile([NC, 4, 128, CLT, 130], bf16, addr_space="Shared",
                        name=f"vg{i}") for i in range(3)]
        rgroups = [list(range(NC))]

        # ---------- projection helper (q / k): [d, t] + norm factors ------
        def qk_projection(w_dram, b_row, g_sb, name):
            raw = acts.tile([128, NH, SC], bf16, tag=f"raw_{name}",
                            name=f"raw_{name}")
            pss = pacc.tile([128, 512], f32, tag="pacc", name=f"pss_{name}")
            for m in range(NH):
                wm = wstr.tile([128, NK, 128], bf16, tag="wm",
                               name=f"wm_{name}_{m}")
                nc.sync.dma_start(wm, w_dram[:, m])
                ps = pp.tile([128, 1024], f32, tag="pp", name=f"pj_{name}_{m}")
                for kk in range(NK):
                    nc.tensor.matmul(
                        ps[:, :SC], wm[:, kk, :], xs[:, kk, :],
                        start=(kk == 0), stop=False)
                # bias via rank-1 update: ps += b_head (x) ones
                nc.tensor.matmul(
                    ps[:, :SC], b_row[:, m * 128:(m + 1) * 128], ones_sc,
                    start=False, stop=True)
                nc.vector.tensor_scalar_mul(raw[:, m, :], ps[:, :SC],
                                            g_sb[:, m:m + 1])
                sq = sqp.tile([128, SC], f32, tag="sq")
                nc.scalar.activation(sq, ps[:, :SC], Act.Square)
                nc.tensor.matmul(pss[0:1, :SC], ones_col, sq,
                                 start=(m == 0), stop=(m == NH - 1))
            r1 = smal.tile([1, SC], f32, tag="r1")
            nc.scalar.activation(r1, pss[0:1, :SC], Act.Sqrt,
                                 scale=constv(1.0 / DIM)[0:1],
                                 bias=constv(EPS)[0:1])
            rr = smal.tile([1, SC], f32, tag="rr")
            nc.vector.reciprocal(rr, r1)
            rrb = psw.tile([128, 512], f32, tag="psw", name=f"rrb_{name}")
            nc.tensor.matmul(rrb[:, :SC], ones_row_f, rr,
                             start=True, stop=True)
            ccr = csrp.tile([128, SC], bf16, tag="ccr")
            ssr = csrp.tile([128, SC], bf16, tag="ssr")
            nc.vector.tensor_mul(ccr, cc, rrb[:, :SC])
            nc.vector.tensor_mul(ssr, ss, rrb[:, :SC])
            return raw, ccr, ssr

        def rope_chunk(raw, ccr, ssr, m, dst_ap, name):
            # dst = raw*ccr + swap_halves(raw)*ssr   (swap via PE matmul).
            # The psum swap result is evacuated to bf16 on ACT (idle in this
            # phase) so all three DVE ops run in the 2x bf16 mode.
            pw = psw.tile([128, 512], f32, tag="psw", name=f"sw_{name}_{m}")
            nc.tensor.matmul(pw[:, :SC], psw_sb, raw[:, m, :],
                             start=True, stop=True)
            pwb = sqp.tile([128, SC], bf16, tag="pwb")
            nc.scalar.copy(pwb, pw[:, :SC])
            m1 = sqp.tile([128, SC], bf16, tag="m1")
            nc.vector.tensor_mul(m1, raw[:, m, :], ccr)
            m2 = sqp.tile([128, SC], bf16, tag="m2")
            nc.vector.tensor_mul(m2, pwb, ssr)
            nc.vector.tensor_add(dst_ap, m1, m2)

        # ---------- K and Q projections back-to-back (PE stays dense;
        # the DVE/ACT rms+rope work of K overlaps Q's matmuls) ----------
        raw_k, ccr_k, ssr_k = qk_projection(wk, bkr_sb, gk_sb, "k")
        raw_q, ccr_q, ssr_q = qk_projection(wq, bqr_sb, gq_sb, "q")
        # prefetch prior KV for the first heads of pass 1 (DMA is idle-ish
        # during the projection phase; pass-1 then starts compute-bound)
        NPT = np_pad // 128
        pre_kv = []
        for h in range(3):
            pkh = kvs.tile([128, np_pad], bf16, tag="kload", name=f"prek{h}")
            nc.sync.dma_start(pkh, priorKT[h])
            pvh = kvs.tile([128, NPT, 130], bf16, tag="vload", name=f"prev{h}")
            nc.sync.dma_start(pvh, priorV2[h])
            pre_kv.append((pkh, pvh))
        # Q ropes first: pass-1 head h can start as soon as qn[:, h] exists,
        # so the remaining (K) ropes run on DVE under pass-1's PE/ACT work.
        qn = acts.tile([128, NH, SC], bf16)
        for m in range(NH):
            rope_chunk(raw_q, ccr_q, ssr_q, m, qn[:, m, :], "q")
        kn = acts.tile([128, NH, SCP], bf16)
        nc.vector.memset(kn, 0.0)
        for m in range(NH):
            rope_chunk(raw_k, ccr_k, ssr_k, m, kn[:, m, :SC], "k")
            if m in (5, 11):
                half = 0 if m == 5 else 1
                for m2 in range(6):
                    nc.sync.dma_start(k_cc[half][m2], kn[:, 6 * half + m2, :])
                nc.gpsimd.collective_compute(
                    "AllGather", Alu.bypass, replica_groups=rgroups,
                    ins=[k_cc[half].opt()], outs=[kg[half].opt()])

        # ---------- V production helper (emitted interleaved with pass 1
        # so its PE work hides under pass-1's ACT-bound exp) -------------
        # vt2[t_part, chunk, head, 130]; col 128 = 1.0 (softmax denominator),
        # pad token rows (beyond chunk width) stay 0 except the ones column.
        vt2 = acts.tile([128, 3, NH, 130], bf16)
        nc.vector.memset(vt2, 0.0)
        nc.vector.memset(vt2[:, :, :, 128:129], 1.0)

        def emit_v_oc(oc):
            wcs = []
            for kk in range(NK):
                wc = wcp.tile([128, 512], bf16, tag="wc", name=f"wv_{oc}_{kk}")
                nc.sync.dma_start(wc, wv[:, kk, oc * 512:(oc + 1) * 512])
                wcs.append(wc)
            for ci, (off, w) in enumerate(CHUNKS):
                pv = pacc.tile([128, 512], f32, tag="pacc",
                               name=f"pv_{oc}_{ci}")
                for kk in range(NK):
                    nc.tensor.matmul(
                        pv[0:w, :], xs[:, kk, off:off + w], wcs[kk],
                        start=(kk == 0), stop=False)
                nc.tensor.matmul(
                    pv[0:w, :], ones_row[:, 0:w],
                    bv_sb[:, oc * 512:(oc + 1) * 512],
                    start=False, stop=True)
                nc.vector.tensor_copy(
                    vt2[0:w, ci, 4 * oc:4 * (oc + 1), 0:128],
                    pv[0:w, :].rearrange("p (h d) -> p h d", h=4))
            # ship this oc's 4 heads (full 128 rows incl. zero padding)
            for hh in range(4):
                h = 4 * oc + hh
                for ci in range(CLT):
                    nc.sync.dma_start(v_cc[oc][hh, :, ci, :], vt2[:, ci, h, :])
            nc.gpsimd.collective_compute(
                "AllGather", Alu.bypass, replica_groups=rgroups,
                ins=[v_cc[oc].opt()], outs=[vg[oc].opt()])

        # ---------- attention ----------
        part1 = outp.tile([128, NH, 3, 130], f32)
        oT = outp.tile([128, NH, SC], bf16)

        def attn_accum(h, lhsT_tiles, v_tiles, n_tiles, phase):
            # single packed psum bank: [s-chunk rows, chunk idx, 129+pad]
            pos = pacc.tile([128, 3, 130], f32, tag="pacc",
                            name=f"po_{phase}_{h}")
            n_mm = 0
            total_mm = n_tiles * 3
            for g0 in range(0, n_tiles, 3):
                gn = min(3, n_tiles - g0)
                # scores for 3 l-tiles packed CONTIGUOUSLY across a 2-bank
                # strip (middle tile's matmul split at the bank boundary) so
                # one ACT exp reads [128, 990] with no per-tile overhead.
                ps = pp.tile([128, 1024], f32, tag="pp",
                             name=f"sc_{phase}_{h}_{g0}")
                esc = escp.tile([128, 990], bf16, tag="esc")
                if gn == 3:
                    nc.tensor.matmul(ps[:, 0:330], lhsT_tiles(g0),
                                     qn[:, h, :], start=True, stop=True)
                    nc.tensor.matmul(ps[:, 330:512], lhsT_tiles(g0 + 1),
                                     qn[:, h, 0:182], start=True, stop=True)
                    nc.tensor.matmul(ps[:, 512:660], lhsT_tiles(g0 + 1),
                                     qn[:, h, 182:330], start=True, stop=True)
                    nc.tensor.matmul(ps[:, 660:990], lhsT_tiles(g0 + 2),
                                     qn[:, h, :], start=True, stop=True)
                    nc.scalar.activation(esc, ps[:, 0:990], Act.Exp,
                                         scale=constv(sm_scale))
                else:
                    for j in range(gn):
                        nc.tensor.matmul(ps[:, 512 * j:512 * j + SC],
                                         lhsT_tiles(g0 + j), qn[:, h, :],
                                         start=True, stop=True)
                        nc.scalar.activation(esc[:, SC * j:SC * (j + 1)],
                                             ps[:, 512 * j:512 * j + SC],
                                             Act.Exp, scale=constv(sm_scale))
                for j in range(gn):
                    lt = g0 + j
                    for ci, (off, w) in enumerate(CHUNKS):
                        # one has_written chain for the whole packed bank
                        nc.tensor.matmul(
                            pos[0:w, ci, 0:129],
                            esc[:, SC * j + off:SC * j + off + w], v_tiles(lt),
                            start=(n_mm == 0), stop=(n_mm == total_mm - 1))
                        n_mm += 1
            if phase == "p":
                nc.vector.tensor_copy(part1[:, h, :, :], pos)
            else:
                nc.vector.tensor_add(part1[:, h, :, :], pos,
                                     part1[:, h, :, :])

        # pass 1: prior KV (overlaps the AllGathers); V-projection chunks are
        # interleaved after the first pass-1 heads so their PE work runs
        # while ACT is busy with pass-1 exps.
        for h in range(NH):
            if h < len(pre_kv):
                pkh, pvh = pre_kv[h]
            else:
                pkh = kvs.tile([128, np_pad], bf16, tag="kload")
                nc.sync.dma_start(pkh, priorKT[h])
                pvh = kvs.tile([128, NPT, 130], bf16, tag="vload")
                nc.sync.dma_start(pvh, priorV2[h])
            attn_accum(
                h,
                lambda lt, pkh=pkh: pkh[:, lt * 128:(lt + 1) * 128],
                lambda lt, pvh=pvh: pvh[:, lt, 0:129],
                NPT, "p")
            if h < 3:
                emit_v_oc(h)

        # prefetch first oc of Wo chunks (used after pass 2)
        wo_t = {}
        for hh in range(NH):
            t = wop.tile([128, 512], bf16, tag="wot", name=f"wo_0_{hh}")
            nc.sync.dma_start(t, wo[:, hh, 0:512])
            wo_t[(0, hh)] = t

        # pass 2: current KV (needs AllGather results)
        for h in range(NH):
            kgh = kvs.tile([128, NC, SCP], bf16, tag="kload")
            nc.sync.dma_start(
                kgh, kg[h // 6][:, h % 6].rearrange("c p t -> p c t"))
            vgh = kvs.tile([128, NC, CLT, 130], bf16, tag="vload")
            nc.sync.dma_start(
                vgh, vg[h // 4][:, h % 4].rearrange("c p lt d -> p c lt d"))
            attn_accum(
                h,
                lambda lt, kgh=kgh: kgh[:, lt // CLT,
                                        (lt % CLT) * 128:(lt % CLT + 1) * 128],
                lambda lt, vgh=vgh: vgh[:, lt // CLT, lt % CLT, 0:129],
                NC * CLT, "c")
            # divide by corrected denominator; transpose to [d, t]
            for ci, (off, w) in enumerate(CHUNKS):
                den = smal.tile([128, 1], f32, tag="den")
                nc.vector.tensor_scalar_add(den[0:w, :],
                                            part1[0:w, h, ci, 128:129],
                                            -float(n_pads))
                rcp = smal.tile([128, 1], f32, tag="rcp")
                nc.vector.reciprocal(rcp[0:w, :], den[0:w, :])
                odiv = sqp.tile([128, 128], f32, tag="odiv")
                nc.vector.tensor_scalar_mul(odiv[0:w, :],
                                            part1[0:w, h, ci, 0:128],
                                            rcp[0:w, 0:1])
                ptr = psw.tile([128, 512], f32, tag="psw",
                               name=f"ptr_{h}_{ci}")
                nc.tensor.transpose(ptr[:, :w], odiv[0:w, :],
                                    ident[0:w, 0:w])
                nc.vector.tensor_copy(oT[:, h, off:off + w], ptr[:, :w])

        # ---------- output projection (oc outer; weights prefetched) ------
        for oc in range(3):
            if oc + 1 < 3:
                for hh in range(NH):
                    t = wop.tile([128, 512], bf16, tag="wot",
                                 name=f"wo_{oc + 1}_{hh}")
                    nc.sync.dma_start(
                        t, wo[:, hh, (oc + 1) * 512:(oc + 2) * 512])
                    wo_t[(oc + 1, hh)] = t
            for ci, (off, w) in enumerate(CHUNKS):
                po = pacc.tile([128, 512], f32, tag="pacc",
                               name=f"pout_{oc}_{ci}")
                for hh in range(NH):
                    nc.tensor.matmul(
                        po[0:w, :], oT[:, hh, off:off + w], wo_t[(oc, hh)],
                        start=(hh == 0), stop=False)
                nc.tensor.matmul(
                    po[0:w, :], ones_row[:, 0:w],
                    bo_sb[:, oc * 512:(oc + 1) * 512],
                    start=False, stop=True)
                ob = sqp.tile([128, 512], f32, tag="ob")
                nc.vector.tensor_copy(ob[0:w, :], po[0:w, :])
                nc.sync.dma_start(
                    out[off:off + w, oc * 512:(oc + 1) * 512], ob[0:w, :])

    nc.compile()
    return nc


def _prep(inputs):
    x = np.asarray(inputs["x"], np.float32)
    freqs_angle = np.asarray(inputs["freqs_angle"], np.float32)
    prior_k = np.asarray(inputs["prior_k"], np.float32)
    prior_v = np.asarray(inputs["prior_v"], np.float32)
    cs = int(np.asarray(inputs["current_start"]))

    block = 3 * FRAME
    block_end = (cs // block + 1) * block
    keep_from = max(0, block_end - 6 * FRAME)
    keep_size = min(cs + S_TOTAL - keep_from, prior_k.shape[0] + S_TOTAL)
    n_prior = keep_size - S_TOTAL
    p0 = prior_k.shape[0] - n_prior
    np_pad = -(-n_prior // 128) * 128
    n_pads = (np_pad - n_prior) + NC * (SCP - SC)

    perm = np.concatenate(
        [h * HD + np.concatenate([np.arange(0, HD, 2), np.arange(1, HD, 2)])
         for h in range(NH)])

    WqT = np.ascontiguousarray(np.asarray(inputs["Wq"], np.float32)[perm].T)
    WkT = np.ascontiguousarray(np.asarray(inputs["Wk"], np.float32)[perm].T)
    WvT = np.ascontiguousarray(np.asarray(inputs["Wv"], np.float32).T)
    WoT = np.ascontiguousarray(np.asarray(inputs["Wo"], np.float32).T)

    # [ki, mh, ko, mi] for q/k; [ki, ko, n] for v/o
    wq2 = np.ascontiguousarray(
        WqT.reshape(NK, 128, NH, 128).transpose(1, 2, 0, 3)).astype(_BF16)
    wk2 = np.ascontiguousarray(
        WkT.reshape(NK, 128, NH, 128).transpose(1, 2, 0, 3)).astype(_BF16)
    wv2 = np.ascontiguousarray(
        WvT.reshape(NK, 128, DIM).transpose(1, 0, 2)).astype(_BF16)
    wo2 = np.ascontiguousarray(
        WoT.reshape(NK, 128, DIM).transpose(1, 0, 2)).astype(_BF16)

    def two(vec, p=None):
        v = np.asarray(vec, np.float32)
        if p is not None:
            v = v[p]
        return np.ascontiguousarray(v.reshape(NH, HD).T)

    gq2 = two(inputs["gq"], perm)
    gk2 = two(inputs["gk"], perm)
    bv1 = np.asarray(inputs["bv"], np.float32).reshape(1, DIM).astype(_BF16)
    bo1 = np.asarray(inputs["bo"], np.float32).reshape(1, DIM).astype(_BF16)
    bqr = np.asarray(inputs["bq"], np.float32)[perm].reshape(1, DIM).astype(_BF16)
    bkr = np.asarray(inputs["bk"], np.float32)[perm].reshape(1, DIM).astype(_BF16)

    pswT = np.zeros((HD, HD), _BF16)
    for r in range(HD):
        pswT[(r + HALF) % HD, r] = 1.0   # lhsT of the half-swap permutation

    theta = _build_theta(freqs_angle, cs)          # [S, 64]
    thetaT = np.ascontiguousarray(theta.T)

    pk = prior_k[p0:].reshape(n_prior, DIM)[:, perm]
    priorKT = np.zeros((DIM, np_pad), np.float32)
    priorKT[:, :n_prior] = pk.T
    priorKT = np.ascontiguousarray(priorKT.reshape(NH, HD, np_pad)).astype(_BF16)
    # prior V pre-tiled [h, p, lt, 130]; col 128 = ones (denominator column)
    NPT = np_pad // 128
    priorV2 = np.zeros((NH, np_pad, 130), np.float32)
    priorV2[:, :n_prior, :HD] = prior_v[p0:].transpose(1, 0, 2)
    priorV2[:, :, 128] = 1.0
    priorV2 = np.ascontiguousarray(
        priorV2.reshape(NH, NPT, 128, 130).transpose(0, 2, 1, 3)).astype(_BF16)

    xT = np.ascontiguousarray(x[0].T).astype(_BF16)              # [DIM, S]

    shared = dict(wq=wq2, wk=wk2, wv=wv2, wo=wo2,
                  gq2=gq2, gk2=gk2, bv1=bv1, bo1=bo1, bqr=bqr, bkr=bkr,
                  pswT=pswT, priorKT=priorKT, priorV2=priorV2)
    in_maps = []
    for c in range(NC):
        m = dict(shared)
        xc = xT[:, c * SC:(c + 1) * SC]                          # [DIM, SC]
        m["xsw"] = np.ascontiguousarray(
            xc.reshape(NK, 128, SC).transpose(1, 0, 2))
        m["thetaT"] = np.ascontiguousarray(thetaT[:, c * SC:(c + 1) * SC])
        in_maps.append(m)
    return in_maps, (n_prior, np_pad, n_pads)


def kernel(**inputs) -> np.ndarray:
    import os
    from concourse.bass_utils import run_bass_kernel_spmd

    in_maps, key = _prep(inputs)
    if key not in _cache:
        _cache[key] = _build_program(*key)
    nc = _cache[key]

    trace = bool(int(os.environ.get("KERNEL_TRACE", "0")))
    try:
        res = run_bass_kernel_spmd(
            nc, in_maps, core_ids=list(range(NC)), trace=trace,
            trace_cores=list(range(NC)) if trace else None)
    except ModuleNotFoundError:
        res = run_bass_kernel_spmd(nc, in_maps, core_ids=list(range(NC)),
                                   trace=False)
    kernel.last_results = res
    outp = np.concatenate([res.results[c]["out"] for c in range(NC)], axis=0)
    return outp[None].astype(np.float32)


# revision 33
# speedup vs baseline: 1.0547x; 1.0199x over previous
"""Trainium2 Bass kernel for nn_CausalSelfAttention_5411658793445.

Sharding: queries (token dim) split 8 ways; K/V projection also token-split,
with the current block's roped K / V exchanged via sliced AllGathers so every
core attends over the full kept KV window (prior cache slice + current block).

Per-core device program (identical SPMD program, per-core data):
  1. fp32 projections of the core's 330-token slice (bf16 matmuls):
     kT [d,t] -> rope -> AllGather (2 head-halves); vT direct [t,d] with the
     softmax ones-column baked in -> AllGather (3 head-quarters); qT [d,t].
  2. RMSNorm (partition-dim sum-of-squares via ones-matmul) + RoPE (pair
     components deinterleaved into partition halves by a host-side weight-row
     permutation; the cross-half combine uses a PE half-swap matmul)
  3. Attention, scores-transposed layout: for each head, l-tiles of 128 kept
     positions, processed in PAIRS: PE scores [l,330]x2 into a 2-bank psum ->
     one strided ACT exp (bf16) -> PE (escT as stationary, s-chunks 128/128/74
     for FWL) x [V | ones-column] accumulating numerator AND denominator in a
     single packed psum bank [<=128, 3, 130].  Zero-padded KV rows contribute
     exactly 1.0 to the denominator each and 0 to the numerator -> one
     constant correction.  Pass 1 (prior KV) overlaps the AllGathers; pass 2
     (current KV) follows.
  4. divide, PE-transpose to [d,t], output projection (prefetched weights),
     + bo -> out rows.
"""

import math
from contextlib import ExitStack

import numpy as np
import ml_dtypes

NC = 8
DIM, NH, HD = 1536, 12, 128
HALF = 64
H, W = 22, 40
FRAME = H * W            # 880
S_TOTAL = 2640
SC = S_TOTAL // NC       # 330
SCP = 384                # padded per-core token count (3*128)
EPS = 1e-6
CT = 22
CH = 21
CW = 21
CLT = SCP // 128         # 3 current l-tiles per source core
NK = DIM // 128          # 12 contraction chunks
CHUNKS = [(0, 128), (128, 128), (256, SC - 256)]   # s-chunks 128/128/74

_BF16 = ml_dtypes.bfloat16
_cache: dict = {}


def _build_theta(freqs_angle, cs):
    start_frame = cs // FRAME
    nf = S_TOTAL // FRAME
    t = freqs_angle[start_frame:start_frame + nf, :CT]
    h = freqs_angle[:H, CT:CT + CH]
    w = freqs_angle[:W, CT + CH:CT + CH + CW]
    tf = np.broadcast_to(t[:, None, None, :], (nf, H, W, CT))
    hf = np.broadcast_to(h[None, :, None, :], (nf, H, W, CH))
    wf = np.broadcast_to(w[None, None, :, :], (nf, H, W, CW))
    return np.concatenate([tf, hf, wf], axis=-1).reshape(nf * H * W, HALF)


def _build_program(n_prior, np_pad, n_pads):
    import concourse.bass as bass  # noqa: F401
    import concourse.tile as tile
    from concourse import bacc, mybir
    from concourse.masks import make_identity

    f32 = mybir.dt.float32
    bf16 = mybir.dt.bfloat16
    Act = mybir.ActivationFunctionType
    Alu = mybir.AluOpType

    NPT = np_pad // 128          # prior l-tiles (21)
    sm_scale = 1.0 / math.sqrt(HD)

    nc = bacc.Bacc("TRN2", target_bir_lowering=False, debug=False,
                   num_devices=NC)

    # host-preswizzled inputs
    xsw = nc.dram_tensor("xsw", [128, NK, SC], bf16, kind="ExternalInput").ap()
    thetaT = nc.dram_tensor("thetaT", [HALF, SC], f32, kind="ExternalInput").ap()
    # wq2/wk2: [ki, mh, ko, mi]; wv2/wo2: [ki, ko, n]
    wq = nc.dram_tensor("wq", [128, NH, NK, 128], bf16, kind="ExternalInput").ap()
    wk = nc.dram_tensor("wk", [128, NH, NK, 128], bf16, kind="ExternalInput").ap()
    wv = nc.dram_tensor("wv", [128, NK, DIM], bf16, kind="ExternalInput").ap()
    wo = nc.dram_tensor("wo", [128, NK, DIM], bf16, kind="ExternalInput").ap()
    gq2 = nc.dram_tensor("gq2", [HD, NH], f32, kind="ExternalInput").ap()
    gk2 = nc.dram_tensor("gk2", [HD, NH], f32, kind="ExternalInput").ap()
    bv1 = nc.dram_tensor("bv1", [1, DIM], bf16, kind="ExternalInput").ap()
    bo1 = nc.dram_tensor("bo1", [1, DIM], bf16, kind="ExternalInput").ap()
    bqr = nc.dram_tensor("bqr", [1, DIM], bf16, kind="ExternalInput").ap()
    bkr = nc.dram_tensor("bkr", [1, DIM], bf16, kind="ExternalInput").ap()
    pswT = nc.dram_tensor("pswT", [HD, HD], bf16, kind="ExternalInput").ap()
    priorKT = nc.dram_tensor("priorKT", [NH, HD, np_pad], bf16,
                             kind="ExternalInput").ap()
    # prior V pre-tiled [h, p, lt, 130] with ones column baked at 128
    priorV2 = nc.dram_tensor("priorV2", [NH, 128, np_pad // 128, 130], bf16,
                             kind="ExternalInput").ap()
    out = nc.dram_tensor("out", [SC, DIM], f32, kind="ExternalOutput").ap()

    with tile.TileContext(nc, trace_sim=False) as tc, ExitStack() as ctx:
        consts = ctx.enter_context(tc.tile_pool(name="consts", bufs=1))
        wstr = ctx.enter_context(tc.tile_pool(name="wstr", bufs=3))
        wcp = ctx.enter_context(tc.tile_pool(name="wcp", bufs=14))
        wop = ctx.enter_context(tc.tile_pool(name="wop", bufs=14))
        xpool = ctx.enter_context(tc.tile_pool(name="xpool", bufs=1))
        acts = ctx.enter_context(tc.tile_pool(name="acts", bufs=1))
        sqp = ctx.enter_context(tc.tile_pool(name="sqp", bufs=2))
        csrp = ctx.enter_context(tc.tile_pool(name="csrp", bufs=2))
        kvs = ctx.enter_context(tc.tile_pool(name="kvs", bufs=3))
        escp = ctx.enter_context(tc.tile_pool(name="escp", bufs=3))
        smal = ctx.enter_context(tc.tile_pool(name="smal", bufs=4))
        outp = ctx.enter_context(tc.tile_pool(name="outp", bufs=1))
        dram = ctx.enter_context(tc.tile_pool(name="dram", bufs=1, space="DRAM"))
        pp = ctx.enter_context(tc.tile_pool(name="pp", bufs=2, space="PSUM"))
        psw = ctx.enter_context(tc.tile_pool(name="psw", bufs=2, space="PSUM"))
        pacc = ctx.enter_context(tc.tile_pool(name="pacc", bufs=2, space="PSUM"))

        # ---------- x (hoisted: the very first DMA issued) ----------
        xs = xpool.tile([128, NK, SC], bf16)
        nc.sync.dma_start(xs, xsw)

        # ---------- constants ----------
        _constv_cache = {}

        def constv(val):
            if val not in _constv_cache:
                t = consts.tile([128, 1], f32, name=f"cv_{len(_constv_cache)}")
                nc.vector.memset(t, val)
                _constv_cache[val] = t
            return _constv_cache[val]

        ident = consts.tile([128, 128], f32)
        make_identity(nc, ident)
        ones_col = consts.tile([128, 1], f32)
        nc.vector.memset(ones_col, 1.0)
        ones_row = consts.tile([1, 128], bf16)
        nc.vector.memset(ones_row, 1.0)
        ones_row_f = consts.tile([1, 128], f32)
        nc.vector.memset(ones_row_f, 1.0)
        psw_sb = consts.tile([HD, HD], bf16)
        nc.sync.dma_start(psw_sb, pswT)
        th2 = consts.tile([128, SC], f32)
        nc.sync.dma_start(th2[0:HALF, :], thetaT)
        nc.sync.dma_start(th2[HALF:128, :], thetaT)
        # CC = [cos; cos], SS = [-sin; sin]
        cc = consts.tile([128, SC], f32)
        ss = consts.tile([128, SC], f32)
        nc.scalar.activation(cc, th2, Act.Sin, bias=constv(math.pi / 2.0))
        nc.scalar.activation(ss[0:HALF, :], th2[0:HALF, :], Act.Sin, scale=constv(-1.0)[0:HALF])
        nc.scalar.activation(ss[HALF:128, :], th2[HALF:128, :], Act.Sin)
        gq_sb = consts.tile([HD, NH], f32)
        gk_sb = consts.tile([HD, NH], f32)
        nc.sync.dma_start(gq_sb, gq2)
        nc.sync.dma_start(gk_sb, gk2)
        bv_sb = consts.tile([1, DIM], bf16)
        bo_sb = consts.tile([1, DIM], bf16)
        nc.sync.dma_start(bv_sb, bv1)
        nc.sync.dma_start(bo_sb, bo1)
        bqr_sb = consts.tile([1, DIM], bf16)
        bkr_sb = consts.tile([1, DIM], bf16)
        nc.sync.dma_start(bqr_sb, bqr)
        nc.sync.dma_start(bkr_sb, bkr)
        ones_sc = consts.tile([1, SC], bf16)
        nc.vector.memset(ones_sc, 1.0)

        # ---------- internal DRAM for collectives ----------
        k_cc = [dram.tile([6, HD, SCP], bf16, name=f"kcc{i}") for i in range(2)]
        kg = [dram.tile([NC, 6, HD, SCP], bf16, addr_space="Shared",
                        name=f"kg{i}") for i in range(2)]
        v_cc = [dram.tile([4, 128, CLT, 130], bf16, name=f"vcc{i}")
                for i in range(3)]
        vg = [dram.tile([NC, 4, 128, CLT, 130], bf16, addr_space="Shared",
                        name=f"vg{i}") for i in range(3)]
        rgroups = [list(range(NC))]

        # ---------- projection helper (q / k): [d, t] + norm factors ------
        def qk_projection(w_dram, b_row, g_sb, name):
            raw = acts.tile([128, NH, SC], bf16, tag=f"raw_{name}",
                            name=f"raw_{name}")
            pss = pacc.tile([128, 512], f32, tag="pacc", name=f"pss_{name}")
            for m in range(NH):
                wm = wstr.tile([128, NK, 128], bf16, tag="wm",
                               name=f"wm_{name}_{m}")
                nc.sync.dma_start(wm, w_dram[:, m])
                ps = pp.tile([128, 1024], f32, tag="pp", name=f"pj_{name}_{m}")
                for kk in range(NK):
                    nc.tensor.matmul(
                        ps[:, :SC], wm[:, kk, :], xs[:, kk, :],
                        start=(kk == 0), stop=False)
                # bias via rank-1 update: ps += b_head (x) ones
                nc.tensor.matmul(
                    ps[:, :SC], b_row[:, m * 128:(m + 1) * 128], ones_sc,
                    start=False, stop=True)
                nc.vector.tensor_scalar_mul(raw[:, m, :], ps[:, :SC],
                                            g_sb[:, m:m + 1])
                sq = sqp.tile([128, SC], f32, tag="sq")
                nc.scalar.activation(sq, ps[:, :SC], Act.Square)
                nc.tensor.matmul(pss[0:1, :SC], ones_col, sq,
                                 start=(m == 0), stop=(m == NH - 1))
            r1 = smal.tile([1, SC], f32, tag="r1")
            nc.scalar.activation(r1, pss[0:1, :SC], Act.Sqrt,
                                 scale=constv(1.0 / DIM)[0:1],
                                 bias=constv(EPS)[0:1])
            rr = smal.tile([1, SC], f32, tag="rr")
            nc.vector.reciprocal(rr, r1)
            rrb = psw.tile([128, 512], f32, tag="psw", name=f"rrb_{name}")
            nc.tensor.matmul(rrb[:, :SC], ones_row_f, rr,
                             start=True, stop=True)
            ccr = csrp.tile([128, SC], bf16, tag="ccr")
            ssr = csrp.tile([128, SC], bf16, tag="ssr")
            nc.vector.tensor_mul(ccr, cc, rrb[:, :SC])
            nc.vector.tensor_mul(ssr, ss, rrb[:, :SC])
            return raw, ccr, ssr

        def rope_chunk(raw, ccr, ssr, m, dst_ap, name):
            # dst = raw*ccr + swap_halves(raw)*ssr   (swap via PE matmul).
            # The psum swap result is evacuated to bf16 on ACT (idle in this
            # phase) so all three DVE ops run in the 2x bf16 mode.
            pw = psw.tile([128, 512], f32, tag="psw", name=f"sw_{name}_{m}")
            nc.tensor.matmul(pw[:, :SC], psw_sb, raw[:, m, :],
                             start=True, stop=True)
            pwb = sqp.tile([128, SC], bf16, tag="pwb")
            nc.scalar.copy(pwb, pw[:, :SC])
            m1 = sqp.tile([128, SC], bf16, tag="m1")
            nc.vector.tensor_mul(m1, raw[:, m, :], ccr)
            m2 = sqp.tile([128, SC], bf16, tag="m2")
            nc.vector.tensor_mul(m2, pwb, ssr)
            nc.vector.tensor_add(dst_ap, m1, m2)

        # ---------- K and Q projections back-to-back (PE stays dense;
        # the DVE/ACT rms+rope work of K overlaps Q's matmuls) ----------
        raw_k, ccr_k, ssr_k = qk_projection(wk, bkr_sb, gk_sb, "k")
        raw_q, ccr_q, ssr_q = qk_projection(wq, bqr_sb, gq_sb, "q")
        # prefetch prior KV for the first heads of pass 1 (DMA is idle-ish
        # during the projection phase; pass-1 then starts compute-bound)
        NPT = np_pad // 128
        pre_kv = []
        for h in range(3):
            pkh = kvs.tile([128, np_pad], bf16, tag="kload", name=f"prek{h}")
            nc.sync.dma_start(pkh, priorKT[h])
            pvh = kvs.tile([128, NPT, 130], bf16, tag="vload", name=f"prev{h}")
            nc.sync.dma_start(pvh, priorV2[h])
            pre_kv.append((pkh, pvh))
        # Rope order chosen so the collective queue runs K0,V0,K1,V1,V2 and
        # every AllGather lands before pass-2 consumes it: K heads 0-5 rope
        # first (-> K0 AG), then all Q ropes (unblocking pass-1), then the
        # remaining K ropes (-> K1 AG) run on DVE under later work.
        kn = acts.tile([128, NH, SCP], bf16)
        nc.vector.memset(kn, 0.0)
        qn = acts.tile([128, NH, SC], bf16)

        def emit_k_half(half):
            for m2 in range(6):
                nc.sync.dma_start(k_cc[half][m2], kn[:, 6 * half + m2, :])
            nc.gpsimd.collective_compute(
                "AllGather", Alu.bypass, replica_groups=rgroups,
                ins=[k_cc[half].opt()], outs=[kg[half].opt()])

        # ---------- V production helper (oc 0 emitted before pass 1, ocs
        # 1/2 interleaved into pass 1 so their PE work hides under the
        # ACT-bound exp stream) -------------------------------------------
        # vt2[t_part, chunk, head, 130]; col 128 = 1.0 (softmax denominator),
        # pad token rows (beyond chunk width) stay 0 except the ones column.
        vt2 = acts.tile([128, 3, NH, 130], bf16)
        nc.vector.memset(vt2, 0.0)
        nc.vector.memset(vt2[:, :, :, 128:129], 1.0)

        def emit_v_oc(oc):
            wcs = []
            for kk in range(NK):
                wc = wcp.tile([128, 512], bf16, tag="wc", name=f"wv_{oc}_{kk}")
                nc.sync.dma_start(wc, wv[:, kk, oc * 512:(oc + 1) * 512])
                wcs.append(wc)
            for ci, (off, w) in enumerate(CHUNKS):
                pv = pacc.tile([128, 512], f32, tag="pacc",
                               name=f"pv_{oc}_{ci}")
                for kk in range(NK):
                    nc.tensor.matmul(
                        pv[0:w, :], xs[:, kk, off:off + w], wcs[kk],
                        start=(kk == 0), stop=False)
                nc.tensor.matmul(
                    pv[0:w, :], ones_row[:, 0:w],
                    bv_sb[:, oc * 512:(oc + 1) * 512],
                    start=False, stop=True)
                nc.vector.tensor_copy(
                    vt2[0:w, ci, 4 * oc:4 * (oc + 1), 0:128],
                    pv[0:w, :].rearrange("p (h d) -> p h d", h=4))
            # ship this oc's 4 heads (full 128 rows incl. zero padding)
            for hh in range(4):
                h = 4 * oc + hh
                for ci in range(CLT):
                    nc.sync.dma_start(v_cc[oc][hh, :, ci, :], vt2[:, ci, h, :])
            nc.gpsimd.collective_compute(
                "AllGather", Alu.bypass, replica_groups=rgroups,
                ins=[v_cc[oc].opt()], outs=[vg[oc].opt()])

        for m in range(6):
            rope_chunk(raw_k, ccr_k, ssr_k, m, kn[:, m, :SC], "k")
        emit_k_half(0)
        for m in range(NH):
            rope_chunk(raw_q, ccr_q, ssr_q, m, qn[:, m, :], "q")
        emit_v_oc(0)
        for m in range(6, NH):
            rope_chunk(raw_k, ccr_k, ssr_k, m, kn[:, m, :SC], "k")
        emit_k_half(1)

        # ---------- attention ----------
        part1 = outp.tile([128, NH, 3, 130], f32)
        oT = outp.tile([128, NH, SC], bf16)

        def attn_accum(h, lhsT_tiles, v_tiles, n_tiles, phase):
            # single packed psum bank: [s-chunk rows, chunk idx, 129+pad]
            pos = pacc.tile([128, 3, 130], f32, tag="pacc",
                            name=f"po_{phase}_{h}")
            n_mm = 0
            total_mm = n_tiles * 3
            for g0 in range(0, n_tiles, 3):
                gn = min(3, n_tiles - g0)
                # scores for 3 l-tiles packed CONTIGUOUSLY across a 2-bank
                # strip (middle tile's matmul split at the bank boundary) so
                # one ACT exp reads [128, 990] with no per-tile overhead.
                ps = pp.tile([128, 1024], f32, tag="pp",
                             name=f"sc_{phase}_{h}_{g0}")
                esc = escp.tile([128, 990], bf16, tag="esc")
                if gn == 3:
                    nc.tensor.matmul(ps[:, 0:330], lhsT_tiles(g0),
                                     qn[:, h, :], start=True, stop=True)
                    nc.tensor.matmul(ps[:, 330:512], lhsT_tiles(g0 + 1),
                                     qn[:, h, 0:182], start=True, stop=True)
                    nc.tensor.matmul(ps[:, 512:660], lhsT_tiles(g0 + 1),
                                     qn[:, h, 182:330], start=True, stop=True)
                    nc.tensor.matmul(ps[:, 660:990], lhsT_tiles(g0 + 2),
                                     qn[:, h, :], start=True, stop=True)
                    nc.scalar.activation(esc, ps[:, 0:990], Act.Exp,
                                         scale=constv(sm_scale))
                else:
                    for j in range(gn):
                        nc.tensor.matmul(ps[:, 512 * j:512 * j + SC],
                                         lhsT_tiles(g0 + j), qn[:, h, :],
                                         start=True, stop=True)
                        nc.scalar.activation(esc[:, SC * j:SC * (j + 1)],
                                             ps[:, 512 * j:512 * j + SC],
                                             Act.Exp, scale=constv(sm_scale))
                for j in range(gn):
                    lt = g0 + j
                    for ci, (off, w) in enumerate(CHUNKS):
                        # one has_written chain for the whole packed bank
                        nc.tensor.matmul(
                            pos[0:w, ci, 0:129],
                            esc[:, SC * j + off:SC * j + off + w], v_tiles(lt),
                            start=(n_mm == 0), stop=(n_mm == total_mm - 1))
                        n_mm += 1
            if phase == "p":
                nc.vector.tensor_copy(part1[:, h, :, :], pos)
            else:
                nc.vector.tensor_add(part1[:, h, :, :], pos,
                                     part1[:, h, :, :])

        # pass 1: prior KV (overlaps the AllGathers); V-projection chunks are
        # interleaved after the first pass-1 heads so their PE work runs
        # while ACT is busy with pass-1 exps.
        for h in range(NH):
            if h < len(pre_kv):
                pkh, pvh = pre_kv[h]
            else:
                pkh = kvs.tile([128, np_pad], bf16, tag="kload")
                nc.sync.dma_start(pkh, priorKT[h])
                pvh = kvs.tile([128, NPT, 130], bf16, tag="vload")
                nc.sync.dma_start(pvh, priorV2[h])
            attn_accum(
                h,
                lambda lt, pkh=pkh: pkh[:, lt * 128:(lt + 1) * 128],
                lambda lt, pvh=pvh: pvh[:, lt, 0:129],
                NPT, "p")
            if h < 2:
                emit_v_oc(h + 1)

        # prefetch first oc of Wo chunks (used after pass 2)
        wo_t = {}
        for hh in range(NH):
            t = wop.tile([128, 512], bf16, tag="wot", name=f"wo_0_{hh}")
            nc.sync.dma_start(t, wo[:, hh, 0:512])
            wo_t[(0, hh)] = t

        # pass 2: current KV (needs AllGather results)
        for h in range(NH):
            kgh = kvs.tile([128, NC, SCP], bf16, tag="kload")
            nc.sync.dma_start(
                kgh, kg[h // 6][:, h % 6].rearrange("c p t -> p c t"))
            vgh = kvs.tile([128, NC, CLT, 130], bf16, tag="vload")
            nc.sync.dma_start(
                vgh, vg[h // 4][:, h % 4].rearrange("c p lt d -> p c lt d"))
            attn_accum(
                h,
                lambda lt, kgh=kgh: kgh[:, lt // CLT,
                                        (lt % CLT) * 128:(lt % CLT + 1) * 128],
                lambda lt, vgh=vgh: vgh[:, lt // CLT, lt % CLT, 0:129],
                NC * CLT, "c")
            # divide by corrected denominator; transpose to [d, t]
            for ci, (off, w) in enumerate(CHUNKS):
                den = smal.tile([128, 1], f32, tag="den")
                nc.vector.tensor_scalar_add(den[0:w, :],
                                            part1[0:w, h, ci, 128:129],
                                            -float(n_pads))
                rcp = smal.tile([128, 1], f32, tag="rcp")
                nc.vector.reciprocal(rcp[0:w, :], den[0:w, :])
                odiv = sqp.tile([128, 128], f32, tag="odiv")
                nc.vector.tensor_scalar_mul(odiv[0:w, :],
                                            part1[0:w, h, ci, 0:128],
                                            rcp[0:w, 0:1])
                ptr = psw.tile([128, 512], f32, tag="psw",
                               name=f"ptr_{h}_{ci}")
                nc.tensor.transpose(ptr[:, :w], odiv[0:w, :],
                                    ident[0:w, 0:w])
                nc.vector.tensor_copy(oT[:, h, off:off + w], ptr[:, :w])

        # ---------- output projection (oc outer; weights prefetched) ------
        for oc in range(3):
            if oc + 1 < 3:
                for hh in range(NH):
                    t = wop.tile([128, 512], bf16, tag="wot",
                                 name=f"wo_{oc + 1}_{hh}")
                    nc.sync.dma_start(
                        t, wo[:, hh, (oc + 1) * 512:(oc + 2) * 512])
                    wo_t[(oc + 1, hh)] = t
            for ci, (off, w) in enumerate(CHUNKS):
                po = pacc.tile([128, 512], f32, tag="pacc",
                               name=f"pout_{oc}_{ci}")
                for hh in range(NH):
                    nc.tensor.matmul(
                        po[0:w, :], oT[:, hh, off:off + w], wo_t[(oc, hh)],
                        start=(hh == 0), stop=False)
                nc.tensor.matmul(
                    po[0:w, :], ones_row[:, 0:w],
                    bo_sb[:, oc * 512:(oc + 1) * 512],
                    start=False, stop=True)
                ob = sqp.tile([128, 512], f32, tag="ob")
                nc.vector.tensor_copy(ob[0:w, :], po[0:w, :])
                nc.sync.dma_start(
                    out[off:off + w, oc * 512:(oc + 1) * 512], ob[0:w, :])

    nc.compile()
    return nc


def _prep(inputs):
    x = np.asarray(inputs["x"], np.float32)
    freqs_angle = np.asarray(inputs["freqs_angle"], np.float32)
    prior_k = np.asarray(inputs["prior_k"], np.float32)
    prior_v = np.asarray(inputs["prior_v"], np.float32)
    cs = int(np.asarray(inputs["current_start"]))

    block = 3 * FRAME
    block_end = (cs // block + 1) * block
    keep_from = max(0, block_end - 6 * FRAME)
    keep_size = min(cs + S_TOTAL - keep_from, prior_k.shape[0] + S_TOTAL)
    n_prior = keep_size - S_TOTAL
    p0 = prior_k.shape[0] - n_prior
    np_pad = -(-n_prior // 128) * 128
    n_pads = (np_pad - n_prior) + NC * (SCP - SC)

    perm = np.concatenate(
        [h * HD + np.concatenate([np.arange(0, HD, 2), np.arange(1, HD, 2)])
         for h in range(NH)])

    WqT = np.ascontiguousarray(np.asarray(inputs["Wq"], np.float32)[perm].T)
    WkT = np.ascontiguousarray(np.asarray(inputs["Wk"], np.float32)[perm].T)
    WvT = np.ascontiguousarray(np.asarray(inputs["Wv"], np.float32).T)
    WoT = np.ascontiguousarray(np.asarray(inputs["Wo"], np.float32).T)

    # [ki, mh, ko, mi] for q/k; [ki, ko, n] for v/o
    wq2 = np.ascontiguousarray(
        WqT.reshape(NK, 128, NH, 128).transpose(1, 2, 0, 3)).astype(_BF16)
    wk2 = np.ascontiguousarray(
        WkT.reshape(NK, 128, NH, 128).transpose(1, 2, 0, 3)).astype(_BF16)
    wv2 = np.ascontiguousarray(
        WvT.reshape(NK, 128, DIM).transpose(1, 0, 2)).astype(_BF16)
    wo2 = np.ascontiguousarray(
        WoT.reshape(NK, 128, DIM).transpose(1, 0, 2)).astype(_BF16)

    def two(vec, p=None):
        v = np.asarray(vec, np.float32)
        if p is not None:
            v = v[p]
        return np.ascontiguousarray(v.reshape(NH, HD).T)

    gq2 = two(inputs["gq"], perm)
    gk2 = two(inputs["gk"], perm)
    bv1 = np.asarray(inputs["bv"], np.float32).reshape(1, DIM).astype(_BF16)
    bo1 = np.asarray(inputs["bo"], np.float32).reshape(1, DIM).astype(_BF16)
    bqr = np.asarray(inputs["bq"], np.float32)[perm].reshape(1, DIM).astype(_BF16)
    bkr = np.asarray(inputs["bk"], np.float32)[perm].reshape(1, DIM).astype(_BF16)

    pswT = np.zeros((HD, HD), _BF16)
    for r in range(HD):
        pswT[(r + HALF) % HD, r] = 1.0   # lhsT of the half-swap permutation

    theta = _build_theta(freqs_angle, cs)          # [S, 64]
    thetaT = np.ascontiguousarray(theta.T)

    pk = prior_k[p0:].reshape(n_prior, DIM)[:, perm]
    priorKT = np.zeros((DIM, np_pad), np.float32)
    priorKT[:, :n_prior] = pk.T
    priorKT = np.ascontiguousarray(priorKT.reshape(NH, HD, np_pad)).astype(_BF16)
    # prior V pre-tiled [h, p, lt, 130]; col 128 = ones (denominator column)
    NPT = np_pad // 128
    priorV2 = np.zeros((NH, np_pad, 130), np.float32)
    priorV2[:, :n_prior, :HD] = prior_v[p0:].transpose(1, 0, 2)
    priorV2[:, :, 128] = 1.0
    priorV2 = np.ascontiguousarray(
        priorV2.reshape(NH, NPT, 128, 130).transpose(0, 2, 1, 3)).astype(_BF16)

    xT = np.ascontiguousarray(x[0].T).astype(_BF16)              # [DIM, S]

    shared = dict(wq=wq2, wk=wk2, wv=wv2, wo=wo2,
                  gq2=gq2, gk2=gk2, bv1=bv1, bo1=bo1, bqr=bqr, bkr=bkr,
                  pswT=pswT, priorKT=priorKT, priorV2=priorV2)
    in_maps = []
    for c in range(NC):
        m = dict(shared)
        xc = xT[:, c * SC:(c + 1) * SC]                          # [DIM, SC]
        m["xsw"] = np.ascontiguousarray(
            xc.reshape(NK, 128, SC).transpose(1, 0, 2))
        m["thetaT"] = np.ascontiguousarray(thetaT[:, c * SC:(c + 1) * SC])
        in_maps.append(m)
    return in_maps, (n_prior, np_pad, n_pads)


def kernel(**inputs) -> np.ndarray:
    import os
    from concourse.bass_utils import run_bass_kernel_spmd

    in_maps, key = _prep(inputs)
    if key not in _cache:
        _cache[key] = _build_program(*key)
    nc = _cache[key]

    trace = bool(int(os.environ.get("KERNEL_TRACE", "0")))
    try:
        res = run_bass_kernel_spmd(
            nc, in_maps, core_ids=list(range(NC)), trace=trace,
            trace_cores=list(range(NC)) if trace else None)
    except ModuleNotFoundError:
        res = run_bass_kernel_spmd(nc, in_maps, core_ids=list(range(NC)),
                                   trace=False)
    kernel.last_results = res
    outp = np.concatenate([res.results[c]["out"] for c in range(NC)], axis=0)
    return outp[None].astype(np.float32)
